# revision 24
# baseline (speedup 1.0000x reference)
"""ViT self-attention (B=32, S=577, D=1024, H=16, Dh=64) on 8 TRN2 NeuronCores.

Sharding: data-parallel over batch — each core gets 4 batch elements, no
collectives.

All matmuls run in bf16 (fp32 operands trigger 2-pass LOW_HIGH emulation on
the PE, measured ~2.8x slower per logical matmul). Weights are loaded and
cast to bf16 once per core and stay resident in SBUF.

Per core, per batch:
  phase 0: DMA X f32 tile, DVE-cast to bf16, PE-transpose to X^T tiles
           [din_p, tok] (4 transposes share one PSUM tile, single evac)
  phase 1: Q^T = Wq^T X^T, K^T = Wk^T X^T (lhsT=W bf16, rhs=X^T bf16; bias
           folded into DVE evac, bf16 out), V natural = X Wv (lhsT=X^T,
           rhs=Wv), stored bf16 with a ones column per head ([V_h | 1] ->
           denominator comes out of the ctx matmul for free)
  phase 2: per head pair (row-packed K=64 matmuls at tile_position
           (0,0)/(64,0)): S^T tile = matmul(lhsT=K^T, rhs=Q^T) -> f32 PSUM;
           P^T = exp(S^T/8) on ACT (bf16); ctx natural = matmul(lhsT=P^T,
           rhs=[V_h|1]) accumulated in PSUM with denominator in col 64;
           DVE: batched recip (5 j-tiles at once) + fused (ctx*recip + bv)
           evac.
  phase 3: DMA out per 128-token tile.
"""

import numpy as np

import concourse.bass as bass
import concourse.mybir as mybir
import concourse.tile as tile
from concourse.bass import ds, ts
from concourse.bass_utils import run_bass_kernel_spmd
from concourse.masks import make_identity

F32 = mybir.dt.float32
BF16 = mybir.dt.bfloat16

# ---------------------------------------------------------------------------
# Wait-legalization patch: this walrus build accepts at most ONE ge-mode sync
# wait per instruction (eq-mode counts as two). Tile's sem assignment attaches
# multi-waits directly to instructions, so hoist extras onto standalone
# EventSemaphore carriers (same engine queue, immediately preceding — identical
# semantics, queue is in-order).
# ---------------------------------------------------------------------------
_ctr = [0]


def _split_waits(insts):
    out = []
    for inst in insts:
        si = inst.sync_info
        if si is not None and si.on_wait:
            waits = list(si.on_wait)
            if len(waits) == 1 and waits[0].wait_mode != "sem-eq-imm":
                move = []
            else:
                move = waits
            for w in move:
                _ctr[0] += 1
                ev = mybir.InstEventSemaphore(
                    name=f"wsplit_{_ctr[0]}", opcode="EventSemaphore",
                    engine=inst.engine, debug=inst.debug, ins=[], outs=[],
                    sync_info=mybir.SyncInfo(on_wait=[w], on_update=[]),
                )
                out.append(ev)
            if move:
                inst.sync_info = mybir.SyncInfo(on_wait=[], on_update=list(si.on_update))
        out.append(inst)
    return out


def _install_waitfix():
    if getattr(tile.TileContext, "_waitfix_installed", False):
        return
    from concourse.vector_clock import ScopedClock

    orig_lower = tile.TileContext._lower_ordered_insts

    def patched_lower(self, ordered):
        for name in list(ordered.keys()):
            ordered[name] = _split_waits(ordered[name])
        return orig_lower(self, ordered)

    def patched_dab(self, tick_clock, wait_clock):
        nc = self.nc
        probe = nc.sync.nop(nofuse=True)
        wait_clock.add_sem_waits(probe.ins, ScopedClock({None: tick_clock.global_clock}))
        si = probe.ins.sync_info
        waits = list(si.on_wait) if si is not None else []
        probe.ins.sync_info = mybir.SyncInfo(
            on_wait=[], on_update=list(si.on_update) if si else []
        )
        for w in waits:
            _ctr[0] += 1
            ev = mybir.InstEventSemaphore(
                name=f"wsplit_dab_{_ctr[0]}", opcode="EventSemaphore",
                engine=mybir.EngineType.SP, debug=probe.ins.debug, ins=[], outs=[],
                sync_info=mybir.SyncInfo(on_wait=[w], on_update=[]),
            )
            nc.sync.add_instruction(ev)
        nc.sync.drain()
        nc.all_engine_barrier()
        assert self.sems is not None
        popped = nc._tile_sem_poison_stack.pop()
        assert popped is self._sem_poison
        nc.clear_and_free_semaphores(list(self.sems.allocated().values()))
        nc.all_engine_barrier()

    tile.TileContext._lower_ordered_insts = patched_lower
    tile.TileContext._drain_and_barrier = patched_dab
    tile.TileContext._waitfix_installed = True


_install_waitfix()

N_CORES = 8
B, S, D = 32, 577, 1024
H, Dh = 16, 64
BPC = B // N_CORES  # batches per core
S_TILES = [(t * 128, min(128, S - t * 128)) for t in range((S + 127) // 128)]  # 5 tiles
ND = D // 128  # 8 din/dout tiles
HPAIRS = H // 2

AF = mybir.ActivationFunctionType
OP = mybir.AluOpType


def build_nc():
    nc = bass.Bass()
    hidden = nc.declare_dram_parameter("hidden", [BPC, S, D], F32, isOutput=False)
    wq = nc.declare_dram_parameter("Wq", [D, D], F32, isOutput=False)
    bq = nc.declare_dram_parameter("bq", [D], F32, isOutput=False)
    wk = nc.declare_dram_parameter("Wk", [D, D], F32, isOutput=False)
    bk = nc.declare_dram_parameter("bk", [D], F32, isOutput=False)
    wv = nc.declare_dram_parameter("Wv", [D, D], F32, isOutput=False)
    bv = nc.declare_dram_parameter("bv", [D], F32, isOutput=False)
    out = nc.declare_dram_parameter("out", [BPC, S, D], F32, isOutput=True)

    with tile.TileContext(nc) as tc:
        with (
            tc.tile_pool(name="singles", bufs=1) as singles,
            tc.tile_pool(name="wst", bufs=2) as wst_pool,
            tc.tile_pool(name="xnat", bufs=2) as xnat_pool,
            tc.tile_pool(name="xc", bufs=2) as xc_pool,
            tc.tile_pool(name="xt", bufs=2) as xt_pool,
            tc.tile_pool(name="qt", bufs=2) as qt_pool,
            tc.tile_pool(name="kt", bufs=2) as kt_pool,
            tc.tile_pool(name="v", bufs=2) as v_pool,
            tc.tile_pool(name="pT", bufs=16) as pT_pool,
            tc.tile_pool(name="ostage", bufs=2) as o_pool,
            tc.tile_pool(name="rc", bufs=8) as rc_pool,
            tc.tile_pool(name="psbig", bufs=3, space="PSUM") as ps_big,
            tc.tile_pool(name="psctx", bufs=2, space="PSUM") as ps_ctx,
        ):
            # --- constants ---
            identity = singles.tile([128, 128], BF16)
            make_identity(nc, identity)
            # per-dout-tile bias columns: bqt[:, m] = bq[128m : 128(m+1)]
            bqt = singles.tile([128, ND], F32)
            bkt = singles.tile([128, ND], F32)
            nc.gpsimd.dma_start(out=bqt, in_=bq[:].rearrange("(m p) -> p m", p=128))
            nc.gpsimd.dma_start(out=bkt, in_=bk[:].rearrange("(m p) -> p m", p=128))
            # bv broadcast to all 128 partitions
            bvb = singles.tile([128, D], F32)
            bv_ap = bv[:]
            nc.gpsimd.dma_start(
                out=bvb,
                in_=bass.AP(tensor=bv_ap.tensor, offset=bv_ap.offset, ap=[[0, 128]] + bv_ap.ap),
            )

            # --- weights: loaded f32 once (gpsimd SWDGE queue — keeps them off
            # the sync queue that feeds X tiles and off the ACT queue that
            # runs the exps), cast to bf16, kept resident ---
            wres = {}

            def emit_wload(wname, wdram):
                tiles = []
                for k in range(ND):
                    wfull = wst_pool.tile([128, D], F32, tag="wst")
                    nc.gpsimd.dma_start(out=wfull, in_=wdram[ts(k, 128), :])
                    wb = singles.tile([128, D], BF16, tag=f"{wname}{k}", name=f"{wname}{k}")
                    nc.vector.tensor_copy(out=wb, in_=wfull)
                    tiles.append(wb)
                wres[wname] = tiles

            # ------------------------------------------------------------------
            # Software-pipelined emission. Per-engine queues are strict FIFO,
            # so program order IS the PE instruction order: interleave batch
            # b's QKV-projection matmuls (dense PE work) between batch b-1's
            # scores (which pace on ACT exp) and ctx matmuls. This keeps the
            # PE MM duty cycle high so the HAM clock gate stays at K=8/8.
            # ------------------------------------------------------------------
            state = {}  # per-batch tiles: xt3, qt3, kt3, vt, ost, ptiles

            def emit_ph0(b):
                st_ = {}
                xt_all = xt_pool.tile([128, ND * S], BF16, tag="xt", name="xt")
                st_["xt3"] = xt3 = xt_all.rearrange("p (k c) -> p k c", c=S)
                for t, (t0, stt) in enumerate(S_TILES):
                    xn = xnat_pool.tile([128, D], F32, tag="xn")
                    # batch 0 is latency-critical at startup: split X loads
                    # across the sync and scalar HWDGE queues
                    eng = nc.scalar if (b == 0 and t % 2) else nc.sync
                    eng.dma_start(out=xn[:stt], in_=hidden[b, t0 : t0 + stt, :])
                    xc = xc_pool.tile([128, D], BF16, tag="xc")
                    nc.vector.tensor_copy(out=xc[:stt], in_=xn[:stt])
                    # 8 bf16 transposes share one PSUM tile, single evac
                    pst = ps_big.tile([128, 1024], BF16, tag="big", name="pstr")
                    for j in range(ND):
                        nc.tensor.transpose(
                            pst[:, 128 * j : 128 * j + stt],
                            xc[:stt, ts(j, 128)],
                            identity[:stt, :stt],
                        )
                    src = pst.rearrange("p (j c) -> p j c", c=128)[:, :, 0:stt]
                    nc.vector.tensor_copy(out=xt3[:, :, t0 : t0 + stt], in_=src)
                qt_all = qt_pool.tile([128, ND * S], BF16, tag="qt", name="qt")
                kt_all = kt_pool.tile([128, ND * S], BF16, tag="kt", name="kt")
                st_["qt3"] = qt_all.rearrange("p (m c) -> p m c", c=S)
                st_["kt3"] = kt_all.rearrange("p (m c) -> p m c", c=S)
                st_["vt"] = [
                    v_pool.tile([128, H * 65], BF16, tag=f"v{t}", name=f"vtile{t}")
                    for t in range(len(S_TILES))
                ]
                st_["ost"] = [
                    o_pool.tile([128, D], F32, tag=f"o{j}", name=f"otile{j}")
                    for j in range(len(S_TILES))
                ]
                st_["ptiles"] = {}
                state[b] = st_

            def emit_proj_unit(b, u):
                st_ = state[b]
                xt3 = st_["xt3"]
                if u < 2 * ND:  # Q (u<8) or K (u<16) dout-tile m
                    wtiles, dst3, bias_t = (
                        (wres["wq"], st_["qt3"], bqt)
                        if u < ND
                        else (wres["wk"], st_["kt3"], bkt)
                    )
                    m = u % ND
                    ps = ps_big.tile([128, 1024], F32, tag="big", name="psbig")
                    for k in range(ND):
                        nc.tensor.matmul(
                            ps[:, 0:512], wtiles[k][:, ts(m, 128)], xt3[:, k, 0:512],
                            start=(k == 0), stop=(k == ND - 1),
                        )
                        nc.tensor.matmul(
                            ps[:, 512:S], wtiles[k][:, ts(m, 128)], xt3[:, k, 512:S],
                            start=(k == 0), stop=(k == ND - 1),
                        )
                    nc.vector.tensor_scalar_add(
                        dst3[:, m, :], ps[:, 0:S], bias_t[:, m : m + 1]
                    )
                else:  # V token-tile t
                    t = u - 2 * ND
                    t0, stt = S_TILES[t]
                    ps = ps_big.tile([128, 1024], F32, tag="big", name="psbig")
                    for k in range(ND):
                        nc.tensor.matmul(
                            ps[:stt, 0:512], xt3[:, k, t0 : t0 + stt], wres["wv"][k][:, 0:512],
                            start=(k == 0), stop=(k == ND - 1),
                        )
                        nc.tensor.matmul(
                            ps[:stt, 512:1024], xt3[:, k, t0 : t0 + stt], wres["wv"][k][:, 512:1024],
                            start=(k == 0), stop=(k == ND - 1),
                        )
                    v3 = st_["vt"][t].rearrange("p (h c) -> p h c", c=65)
                    nc.vector.tensor_copy(
                        out=v3[:stt, :, 0:64],
                        in_=ps[:stt].rearrange("p (h c) -> p h c", c=64),
                    )
                    nc.vector.memset(v3[:, :, 64:65], 1.0)

            def emit_scores(b, p):
                st_ = state[b]
                qt3, kt3 = st_["qt3"], st_["kt3"]
                ptiles = ([], [])
                for t, (t0, stt) in enumerate(S_TILES):
                    for half in range(2):
                        h0 = half * 64
                        psS = ps_big.tile([128, 1024], F32, tag="big", name="psbig")
                        nc.tensor.matmul(
                            psS[:stt, 0:512],
                            kt3[h0 : h0 + 64, p, t0 : t0 + stt],
                            qt3[h0 : h0 + 64, p, 0:512],
                            start=True, stop=True, tile_position=(h0, 0),
                        )
                        nc.tensor.matmul(
                            psS[:stt, 512:S],
                            kt3[h0 : h0 + 64, p, t0 : t0 + stt],
                            qt3[h0 : h0 + 64, p, 512:S],
                            start=True, stop=True, tile_position=(h0, 0),
                        )
                        pT = pT_pool.tile([128, S], BF16, tag="pT", name="pTtile")
                        nc.scalar.activation(pT[:stt], psS[:stt, 0:S], AF.Exp, scale=0.125)
                        ptiles[half].append(pT)
                st_["ptiles"][p] = ptiles

            def emit_ctx(b, p):
                st_ = state[b]
                ptiles = st_["ptiles"].pop(p)
                vt, ost = st_["vt"], st_["ost"]
                for half in range(2):
                    h = 2 * p + half
                    psc = ps_ctx.tile([128, 512], F32, tag="ctx", name="psctx")
                    for j, (j0, sj) in enumerate(S_TILES):
                        for t, (t0, stt) in enumerate(S_TILES):
                            nc.tensor.matmul(
                                psc[:sj, ds(65 * j, 65)],
                                ptiles[half][t][:stt, j0 : j0 + sj],
                                vt[t][:stt, ds(65 * h, 65)],
                                start=(t == 0), stop=(t == len(S_TILES) - 1),
                            )
                    # batched reciprocal of the 5 denominator columns
                    rc = rc_pool.tile([128, 8], F32, tag="rc", name="rctile")
                    psc3 = psc[:, 0:325].rearrange("p (j c) -> p j c", c=65)
                    rc3 = rc.rearrange("p (j c) -> p j c", c=1)
                    nc.vector.reciprocal(rc3[:, 0:5, :], psc3[:, 0:5, 64:65])
                    for j, (j0, sj) in enumerate(S_TILES):
                        nc.vector.scalar_tensor_tensor(
                            out=ost[j][:sj, ds(64 * h, 64)],
                            in0=psc[:sj, ds(65 * j, 64)],
                            scalar=rc[:sj, j : j + 1],
                            in1=bvb[:sj, ds(64 * h, 64)],
                            op0=OP.mult,
                            op1=OP.add,
                        )

            def emit_store(b):
                # gpsimd SWDGE queue: keeps the sync queue free for the next
                # batch's X loads (stores ahead of X loads in one FIFO caused
                # ~7us PE stalls at every batch boundary)
                ost = state[b]["ost"]
                for j, (j0, sj) in enumerate(S_TILES):
                    nc.gpsimd.dma_start(out=out[b, j0 : j0 + sj, :], in_=ost[j][:sj])
                del state[b]

            NU = 2 * ND + len(S_TILES)  # 21 projection units per batch
            NSLOT = HPAIRS + 1  # pair slots incl. ctx flush
            emit_ph0(0)
            emit_wload("wq", wq)
            emit_wload("wk", wk)
            emit_wload("wv", wv)
            for b in range(BPC + 1):
                for p in range(NSLOT):
                    if b >= 1 and p < HPAIRS:
                        emit_scores(b - 1, p)
                    if b < BPC:
                        for u in range(NU * p // NSLOT, NU * (p + 1) // NSLOT):
                            emit_proj_unit(b, u)
                    if p == 5 and b + 1 < BPC:
                        # hoist next batch's load/cast/transpose: its DVE work
                        # queues ahead of this iteration's tail ctx evacs, so
                        # the PE transposes aren't left waiting at the boundary
                        emit_ph0(b + 1)
                    if b >= 1 and p >= 1:
                        emit_ctx(b - 1, p - 1)
                if b >= 1:
                    emit_store(b - 1)

    return nc


_NC = None


def kernel(hidden_states, Wq, bq, Wk, bk, Wv, bv):
    global _NC
    if _NC is None:
        _NC = build_nc()
    hs = np.ascontiguousarray(np.asarray(hidden_states, dtype=np.float32))
    args = {
        "Wq": np.ascontiguousarray(np.asarray(Wq, np.float32)),
        "bq": np.ascontiguousarray(np.asarray(bq, np.float32)),
        "Wk": np.ascontiguousarray(np.asarray(Wk, np.float32)),
        "bk": np.ascontiguousarray(np.asarray(bk, np.float32)),
        "Wv": np.ascontiguousarray(np.asarray(Wv, np.float32)),
        "bv": np.ascontiguousarray(np.asarray(bv, np.float32)),
    }
    in_maps = [
        {"hidden": hs[i * BPC : (i + 1) * BPC], **args} for i in range(N_CORES)
    ]
    res = run_bass_kernel_spmd(_NC, in_maps, list(range(N_CORES)))
    return np.concatenate([res.results[i]["out"] for i in range(N_CORES)], axis=0)


# revision 27
# speedup vs baseline: 1.0205x; 1.0205x over previous
"""ViT self-attention (B=32, S=577, D=1024, H=16, Dh=64) on 8 TRN2 NeuronCores.

Sharding: data-parallel over batch — each core gets 4 batch elements, no
collectives.

All matmuls run in bf16 (fp32 operands trigger 2-pass LOW_HIGH emulation on
the PE, measured ~2.8x slower per logical matmul). Weights are loaded and
cast to bf16 once per core and stay resident in SBUF.

Per core, per batch:
  phase 0: DMA X f32 tile, DVE-cast to bf16, PE-transpose to X^T tiles
           [din_p, tok] (4 transposes share one PSUM tile, single evac)
  phase 1: Q^T = Wq^T X^T, K^T = Wk^T X^T (lhsT=W bf16, rhs=X^T bf16; bias
           folded into DVE evac, bf16 out), V natural = X Wv (lhsT=X^T,
           rhs=Wv), stored bf16 with a ones column per head ([V_h | 1] ->
           denominator comes out of the ctx matmul for free)
  phase 2: per head pair (row-packed K=64 matmuls at tile_position
           (0,0)/(64,0)): S^T tile = matmul(lhsT=K^T, rhs=Q^T) -> f32 PSUM;
           P^T = exp(S^T/8) on ACT (bf16); ctx natural = matmul(lhsT=P^T,
           rhs=[V_h|1]) accumulated in PSUM with denominator in col 64;
           DVE: batched recip (5 j-tiles at once) + fused (ctx*recip + bv)
           evac.
  phase 3: DMA out per 128-token tile.
"""

import numpy as np

import concourse.bass as bass
import concourse.mybir as mybir
import concourse.tile as tile
from concourse.bass import ds, ts
from concourse.bass_utils import run_bass_kernel_spmd
from concourse.masks import make_identity

F32 = mybir.dt.float32
BF16 = mybir.dt.bfloat16

# ---------------------------------------------------------------------------
# Wait-legalization patch: this walrus build accepts at most ONE ge-mode sync
# wait per instruction (eq-mode counts as two). Tile's sem assignment attaches
# multi-waits directly to instructions, so hoist extras onto standalone
# EventSemaphore carriers (same engine queue, immediately preceding — identical
# semantics, queue is in-order).
# ---------------------------------------------------------------------------
_ctr = [0]


def _split_waits(insts):
    out = []
    for inst in insts:
        si = inst.sync_info
        if si is not None and si.on_wait:
            waits = list(si.on_wait)
            if len(waits) == 1 and waits[0].wait_mode != "sem-eq-imm":
                move = []
            else:
                move = waits
            for w in move:
                _ctr[0] += 1
                ev = mybir.InstEventSemaphore(
                    name=f"wsplit_{_ctr[0]}", opcode="EventSemaphore",
                    engine=inst.engine, debug=inst.debug, ins=[], outs=[],
                    sync_info=mybir.SyncInfo(on_wait=[w], on_update=[]),
                )
                out.append(ev)
            if move:
                inst.sync_info = mybir.SyncInfo(on_wait=[], on_update=list(si.on_update))
        out.append(inst)
    return out


def _install_waitfix():
    if getattr(tile.TileContext, "_waitfix_installed", False):
        return
    from concourse.vector_clock import ScopedClock

    orig_lower = tile.TileContext._lower_ordered_insts

    def patched_lower(self, ordered):
        for name in list(ordered.keys()):
            ordered[name] = _split_waits(ordered[name])
        return orig_lower(self, ordered)

    def patched_dab(self, tick_clock, wait_clock):
        nc = self.nc
        probe = nc.sync.nop(nofuse=True)
        wait_clock.add_sem_waits(probe.ins, ScopedClock({None: tick_clock.global_clock}))
        si = probe.ins.sync_info
        waits = list(si.on_wait) if si is not None else []
        probe.ins.sync_info = mybir.SyncInfo(
            on_wait=[], on_update=list(si.on_update) if si else []
        )
        for w in waits:
            _ctr[0] += 1
            ev = mybir.InstEventSemaphore(
                name=f"wsplit_dab_{_ctr[0]}", opcode="EventSemaphore",
                engine=mybir.EngineType.SP, debug=probe.ins.debug, ins=[], outs=[],
                sync_info=mybir.SyncInfo(on_wait=[w], on_update=[]),
            )
            nc.sync.add_instruction(ev)
        nc.sync.drain()
        nc.all_engine_barrier()
        assert self.sems is not None
        popped = nc._tile_sem_poison_stack.pop()
        assert popped is self._sem_poison
        nc.clear_and_free_semaphores(list(self.sems.allocated().values()))
        nc.all_engine_barrier()

    tile.TileContext._lower_ordered_insts = patched_lower
    tile.TileContext._drain_and_barrier = patched_dab
    tile.TileContext._waitfix_installed = True


_install_waitfix()

N_CORES = 8
B, S, D = 32, 577, 1024
H, Dh = 16, 64
BPC = B // N_CORES  # batches per core
S_TILES = [(t * 128, min(128, S - t * 128)) for t in range((S + 127) // 128)]  # 5 tiles
ND = D // 128  # 8 din/dout tiles
HPAIRS = H // 2

AF = mybir.ActivationFunctionType
OP = mybir.AluOpType


def build_nc():
    nc = bass.Bass()
    hidden = nc.declare_dram_parameter("hidden", [BPC, S, D], F32, isOutput=False)
    wq = nc.declare_dram_parameter("Wq", [D, D], F32, isOutput=False)
    bq = nc.declare_dram_parameter("bq", [D], F32, isOutput=False)
    wk = nc.declare_dram_parameter("Wk", [D, D], F32, isOutput=False)
    bk = nc.declare_dram_parameter("bk", [D], F32, isOutput=False)
    wv = nc.declare_dram_parameter("Wv", [D, D], F32, isOutput=False)
    bv = nc.declare_dram_parameter("bv", [D], F32, isOutput=False)
    out = nc.declare_dram_parameter("out", [BPC, S, D], F32, isOutput=True)

    with tile.TileContext(nc) as tc:
        with (
            tc.tile_pool(name="singles", bufs=1) as singles,
            tc.tile_pool(name="wst", bufs=2) as wst_pool,
            tc.tile_pool(name="xnat", bufs=2) as xnat_pool,
            tc.tile_pool(name="xc", bufs=2) as xc_pool,
            tc.tile_pool(name="xt", bufs=2) as xt_pool,
            tc.tile_pool(name="qt", bufs=2) as qt_pool,
            tc.tile_pool(name="kt", bufs=2) as kt_pool,
            tc.tile_pool(name="v", bufs=2) as v_pool,
            tc.tile_pool(name="pT", bufs=16) as pT_pool,
            tc.tile_pool(name="ostage", bufs=2) as o_pool,
            tc.tile_pool(name="rc", bufs=8) as rc_pool,
            tc.tile_pool(name="pssc", bufs=2, space="PSUM") as ps_sc,
            tc.tile_pool(name="pspr", bufs=1, space="PSUM") as ps_pr,
            tc.tile_pool(name="psctx", bufs=2, space="PSUM") as ps_ctx,
        ):
            # --- constants ---
            identity = singles.tile([128, 128], BF16)
            make_identity(nc, identity)
            # per-dout-tile bias columns: bqt[:, m] = bq[128m : 128(m+1)]
            bqt = singles.tile([128, ND], F32)
            bkt = singles.tile([128, ND], F32)
            nc.gpsimd.dma_start(out=bqt, in_=bq[:].rearrange("(m p) -> p m", p=128))
            nc.gpsimd.dma_start(out=bkt, in_=bk[:].rearrange("(m p) -> p m", p=128))
            # bv broadcast to all 128 partitions
            bvb = singles.tile([128, D], F32)
            bv_ap = bv[:]
            nc.gpsimd.dma_start(
                out=bvb,
                in_=bass.AP(tensor=bv_ap.tensor, offset=bv_ap.offset, ap=[[0, 128]] + bv_ap.ap),
            )

            # --- weights: loaded f32 once (gpsimd SWDGE queue — keeps them off
            # the sync queue that feeds X tiles and off the ACT queue that
            # runs the exps), cast to bf16, kept resident ---
            wres = {}

            def emit_wload(wname, wdram):
                tiles = []
                for k in range(ND):
                    wfull = wst_pool.tile([128, D], F32, tag="wst")
                    nc.gpsimd.dma_start(out=wfull, in_=wdram[ts(k, 128), :])
                    wb = singles.tile([128, D], BF16, tag=f"{wname}{k}", name=f"{wname}{k}")
                    nc.vector.tensor_copy(out=wb, in_=wfull)
                    tiles.append(wb)
                wres[wname] = tiles

            # ------------------------------------------------------------------
            # Software-pipelined emission. Per-engine queues are strict FIFO,
            # so program order IS the PE instruction order: interleave batch
            # b's QKV-projection matmuls (dense PE work) between batch b-1's
            # scores (which pace on ACT exp) and ctx matmuls. This keeps the
            # PE MM duty cycle high so the HAM clock gate stays at K=8/8.
            # ------------------------------------------------------------------
            state = {}  # per-batch tiles: xt3, qt3, kt3, vt, ost, ptiles

            def emit_ph0(b):
                st_ = {}
                xt_all = xt_pool.tile([128, ND * S], BF16, tag="xt", name="xt")
                st_["xt3"] = xt3 = xt_all.rearrange("p (k c) -> p k c", c=S)
                for t, (t0, stt) in enumerate(S_TILES):
                    xn = xnat_pool.tile([128, D], F32, tag="xn")
                    # batch 0 is latency-critical at startup: split X loads
                    # across the sync and scalar HWDGE queues
                    eng = nc.scalar if (b == 0 and t % 2) else nc.sync
                    eng.dma_start(out=xn[:stt], in_=hidden[b, t0 : t0 + stt, :])
                    xc = xc_pool.tile([128, D], BF16, tag="xc")
                    nc.vector.tensor_copy(out=xc[:stt], in_=xn[:stt])
                    # 8 bf16 transposes share one PSUM tile, single evac
                    pst = ps_pr.tile([128, 1024], BF16, tag="pr", name="pstr")
                    for j in range(ND):
                        nc.tensor.transpose(
                            pst[:, 128 * j : 128 * j + stt],
                            xc[:stt, ts(j, 128)],
                            identity[:stt, :stt],
                        )
                    src = pst.rearrange("p (j c) -> p j c", c=128)[:, :, 0:stt]
                    nc.vector.tensor_copy(out=xt3[:, :, t0 : t0 + stt], in_=src)
                qt_all = qt_pool.tile([128, ND * S], BF16, tag="qt", name="qt")
                kt_all = kt_pool.tile([128, ND * S], BF16, tag="kt", name="kt")
                st_["qt3"] = qt_all.rearrange("p (m c) -> p m c", c=S)
                st_["kt3"] = kt_all.rearrange("p (m c) -> p m c", c=S)
                st_["vt"] = [
                    v_pool.tile([128, H * 65], BF16, tag=f"v{t}", name=f"vtile{t}")
                    for t in range(len(S_TILES))
                ]
                st_["ost"] = [
                    o_pool.tile([128, D], F32, tag=f"o{j}", name=f"otile{j}")
                    for j in range(len(S_TILES))
                ]
                st_["ptiles"] = {}
                state[b] = st_

            def emit_proj_unit(b, u):
                st_ = state[b]
                xt3 = st_["xt3"]
                if u < 2 * ND:  # Q (u<8) or K (u<16) dout-tile m
                    wtiles, dst3, bias_t = (
                        (wres["wq"], st_["qt3"], bqt)
                        if u < ND
                        else (wres["wk"], st_["kt3"], bkt)
                    )
                    m = u % ND
                    ps = ps_pr.tile([128, 1024], F32, tag="pr", name="pspr")
                    for k in range(ND):
                        nc.tensor.matmul(
                            ps[:, 0:512], wtiles[k][:, ts(m, 128)], xt3[:, k, 0:512],
                            start=(k == 0), stop=(k == ND - 1),
                        )
                        nc.tensor.matmul(
                            ps[:, 512:S], wtiles[k][:, ts(m, 128)], xt3[:, k, 512:S],
                            start=(k == 0), stop=(k == ND - 1),
                        )
                    nc.vector.tensor_scalar_add(
                        dst3[:, m, :], ps[:, 0:S], bias_t[:, m : m + 1]
                    )
                else:  # V token-tile t
                    t = u - 2 * ND
                    t0, stt = S_TILES[t]
                    ps = ps_pr.tile([128, 1024], F32, tag="pr", name="pspr")
                    for k in range(ND):
                        nc.tensor.matmul(
                            ps[:stt, 0:512], xt3[:, k, t0 : t0 + stt], wres["wv"][k][:, 0:512],
                            start=(k == 0), stop=(k == ND - 1),
                        )
                        nc.tensor.matmul(
                            ps[:stt, 512:1024], xt3[:, k, t0 : t0 + stt], wres["wv"][k][:, 512:1024],
                            start=(k == 0), stop=(k == ND - 1),
                        )
                    v3 = st_["vt"][t].rearrange("p (h c) -> p h c", c=65)
                    nc.vector.tensor_copy(
                        out=v3[:stt, :, 0:64],
                        in_=ps[:stt].rearrange("p (h c) -> p h c", c=64),
                    )
                    nc.vector.memset(v3[:, :, 64:65], 1.0)

            def emit_scores(b, p):
                st_ = state[b]
                qt3, kt3 = st_["qt3"], st_["kt3"]
                ptiles = ([], [])
                for t, (t0, stt) in enumerate(S_TILES):
                    for half in range(2):
                        h0 = half * 64
                        psS = ps_sc.tile([128, 1024], F32, tag="sc", name="pssc")
                        nc.tensor.matmul(
                            psS[:stt, 0:512],
                            kt3[h0 : h0 + 64, p, t0 : t0 + stt],
                            qt3[h0 : h0 + 64, p, 0:512],
                            start=True, stop=True, tile_position=(h0, 0),
                        )
                        nc.tensor.matmul(
                            psS[:stt, 512:S],
                            kt3[h0 : h0 + 64, p, t0 : t0 + stt],
                            qt3[h0 : h0 + 64, p, 512:S],
                            start=True, stop=True, tile_position=(h0, 0),
                        )
                        pT = pT_pool.tile([128, S], BF16, tag="pT", name="pTtile")
                        nc.scalar.activation(pT[:stt], psS[:stt, 0:S], AF.Exp, scale=0.125)
                        ptiles[half].append(pT)
                st_["ptiles"][p] = ptiles

            def emit_ctx(b, p):
                st_ = state[b]
                ptiles = st_["ptiles"].pop(p)
                vt, ost = st_["vt"], st_["ost"]
                for half in range(2):
                    h = 2 * p + half
                    psc = ps_ctx.tile([128, 512], F32, tag="ctx", name="psctx")
                    for j, (j0, sj) in enumerate(S_TILES):
                        for t, (t0, stt) in enumerate(S_TILES):
                            nc.tensor.matmul(
                                psc[:sj, ds(65 * j, 65)],
                                ptiles[half][t][:stt, j0 : j0 + sj],
                                vt[t][:stt, ds(65 * h, 65)],
                                start=(t == 0), stop=(t == len(S_TILES) - 1),
                            )
                    # batched reciprocal of the 5 denominator columns
                    rc = rc_pool.tile([128, 8], F32, tag="rc", name="rctile")
                    psc3 = psc[:, 0:325].rearrange("p (j c) -> p j c", c=65)
                    rc3 = rc.rearrange("p (j c) -> p j c", c=1)
                    nc.vector.reciprocal(rc3[:, 0:5, :], psc3[:, 0:5, 64:65])
                    for j, (j0, sj) in enumerate(S_TILES):
                        nc.vector.scalar_tensor_tensor(
                            out=ost[j][:sj, ds(64 * h, 64)],
                            in0=psc[:sj, ds(65 * j, 64)],
                            scalar=rc[:sj, j : j + 1],
                            in1=bvb[:sj, ds(64 * h, 64)],
                            op0=OP.mult,
                            op1=OP.add,
                        )

            def emit_store(b):
                # gpsimd SWDGE queue: keeps the sync queue free for the next
                # batch's X loads (stores ahead of X loads in one FIFO caused
                # ~7us PE stalls at every batch boundary)
                ost = state[b]["ost"]
                for j, (j0, sj) in enumerate(S_TILES):
                    nc.gpsimd.dma_start(out=out[b, j0 : j0 + sj, :], in_=ost[j][:sj])
                del state[b]

            NU = 2 * ND + len(S_TILES)  # 21 projection units per batch
            NSLOT = HPAIRS + 1  # pair slots incl. ctx flush
            emit_ph0(0)
            emit_wload("wq", wq)
            emit_wload("wk", wk)
            emit_wload("wv", wv)
            for b in range(BPC + 1):
                for p in range(NSLOT):
                    if b >= 1 and p < HPAIRS:
                        emit_scores(b - 1, p)
                    if b < BPC:
                        for u in range(NU * p // NSLOT, NU * (p + 1) // NSLOT):
                            emit_proj_unit(b, u)
                    if p == 5 and b + 1 < BPC:
                        # hoist next batch's load/cast/transpose: its DVE work
                        # queues ahead of this iteration's tail ctx evacs, so
                        # the PE transposes aren't left waiting at the boundary
                        emit_ph0(b + 1)
                    if b >= 1 and p >= 1:
                        emit_ctx(b - 1, p - 1)
                if b >= 1:
                    emit_store(b - 1)

    return nc


_NC = None


def kernel(hidden_states, Wq, bq, Wk, bk, Wv, bv):
    global _NC
    if _NC is None:
        _NC = build_nc()
    hs = np.ascontiguousarray(np.asarray(hidden_states, dtype=np.float32))
    args = {
        "Wq": np.ascontiguousarray(np.asarray(Wq, np.float32)),
        "bq": np.ascontiguousarray(np.asarray(bq, np.float32)),
        "Wk": np.ascontiguousarray(np.asarray(Wk, np.float32)),
        "bk": np.ascontiguousarray(np.asarray(bk, np.float32)),
        "Wv": np.ascontiguousarray(np.asarray(Wv, np.float32)),
        "bv": np.ascontiguousarray(np.asarray(bv, np.float32)),
    }
    in_maps = [
        {"hidden": hs[i * BPC : (i + 1) * BPC], **args} for i in range(N_CORES)
    ]
    res = run_bass_kernel_spmd(_NC, in_maps, list(range(N_CORES)))
    return np.concatenate([res.results[i]["out"] for i in range(N_CORES)], axis=0)


# revision 28
# speedup vs baseline: 1.0789x; 1.0572x over previous
"""ViT self-attention (B=32, S=577, D=1024, H=16, Dh=64) on 8 TRN2 NeuronCores.

Sharding: data-parallel over batch — each core gets 4 batch elements, no
collectives.

All matmuls run in bf16 (fp32 operands trigger 2-pass LOW_HIGH emulation on
the PE, measured ~2.8x slower per logical matmul). Weights are loaded and
cast to bf16 once per core and stay resident in SBUF.

Per core, per batch:
  phase 0: DMA X f32 tile, DVE-cast to bf16, PE-transpose to X^T tiles
           [din_p, tok] (4 transposes share one PSUM tile, single evac)
  phase 1: Q^T = Wq^T X^T, K^T = Wk^T X^T (lhsT=W bf16, rhs=X^T bf16; bias
           folded into DVE evac, bf16 out), V natural = X Wv (lhsT=X^T,
           rhs=Wv), stored bf16 with a ones column per head ([V_h | 1] ->
           denominator comes out of the ctx matmul for free)
  phase 2: per head pair (row-packed K=64 matmuls at tile_position
           (0,0)/(64,0)): S^T tile = matmul(lhsT=K^T, rhs=Q^T) -> f32 PSUM;
           P^T = exp(S^T/8) on ACT (bf16); ctx natural = matmul(lhsT=P^T,
           rhs=[V_h|1]) accumulated in PSUM with denominator in col 64;
           DVE: batched recip (5 j-tiles at once) + fused (ctx*recip + bv)
           evac.
  phase 3: DMA out per 128-token tile.
"""

import numpy as np

import concourse.bass as bass
import concourse.mybir as mybir
import concourse.tile as tile
from concourse.bass import ds, ts
from concourse.bass_utils import run_bass_kernel_spmd
from concourse.masks import make_identity

F32 = mybir.dt.float32
BF16 = mybir.dt.bfloat16

# ---------------------------------------------------------------------------
# Wait-legalization patch: this walrus build accepts at most ONE ge-mode sync
# wait per instruction (eq-mode counts as two). Tile's sem assignment attaches
# multi-waits directly to instructions, so hoist extras onto standalone
# EventSemaphore carriers (same engine queue, immediately preceding — identical
# semantics, queue is in-order).
# ---------------------------------------------------------------------------
_ctr = [0]


def _split_waits(insts):
    out = []
    for inst in insts:
        si = inst.sync_info
        if si is not None and si.on_wait:
            waits = list(si.on_wait)
            if len(waits) == 1 and waits[0].wait_mode != "sem-eq-imm":
                move = []
            else:
                move = waits
            for w in move:
                _ctr[0] += 1
                ev = mybir.InstEventSemaphore(
                    name=f"wsplit_{_ctr[0]}", opcode="EventSemaphore",
                    engine=inst.engine, debug=inst.debug, ins=[], outs=[],
                    sync_info=mybir.SyncInfo(on_wait=[w], on_update=[]),
                )
                out.append(ev)
            if move:
                inst.sync_info = mybir.SyncInfo(on_wait=[], on_update=list(si.on_update))
        out.append(inst)
    return out


def _install_waitfix():
    if getattr(tile.TileContext, "_waitfix_installed", False):
        return
    from concourse.vector_clock import ScopedClock

    orig_lower = tile.TileContext._lower_ordered_insts

    def patched_lower(self, ordered):
        for name in list(ordered.keys()):
            ordered[name] = _split_waits(ordered[name])
        return orig_lower(self, ordered)

    def patched_dab(self, tick_clock, wait_clock):
        nc = self.nc
        probe = nc.sync.nop(nofuse=True)
        wait_clock.add_sem_waits(probe.ins, ScopedClock({None: tick_clock.global_clock}))
        si = probe.ins.sync_info
        waits = list(si.on_wait) if si is not None else []
        probe.ins.sync_info = mybir.SyncInfo(
            on_wait=[], on_update=list(si.on_update) if si else []
        )
        for w in waits:
            _ctr[0] += 1
            ev = mybir.InstEventSemaphore(
                name=f"wsplit_dab_{_ctr[0]}", opcode="EventSemaphore",
                engine=mybir.EngineType.SP, debug=probe.ins.debug, ins=[], outs=[],
                sync_info=mybir.SyncInfo(on_wait=[w], on_update=[]),
            )
            nc.sync.add_instruction(ev)
        nc.sync.drain()
        nc.all_engine_barrier()
        assert self.sems is not None
        popped = nc._tile_sem_poison_stack.pop()
        assert popped is self._sem_poison
        nc.clear_and_free_semaphores(list(self.sems.allocated().values()))
        nc.all_engine_barrier()

    tile.TileContext._lower_ordered_insts = patched_lower
    tile.TileContext._drain_and_barrier = patched_dab
    tile.TileContext._waitfix_installed = True


_install_waitfix()

N_CORES = 8
B, S, D = 32, 577, 1024
H, Dh = 16, 64
BPC = B // N_CORES  # batches per core
S_TILES = [(t * 128, min(128, S - t * 128)) for t in range((S + 127) // 128)]  # 5 tiles
ND = D // 128  # 8 din/dout tiles
HPAIRS = H // 2

AF = mybir.ActivationFunctionType
OP = mybir.AluOpType


def build_nc():
    nc = bass.Bass()
    hidden = nc.declare_dram_parameter("hidden", [BPC, S, D], F32, isOutput=False)
    wq = nc.declare_dram_parameter("Wq", [D, D], F32, isOutput=False)
    bq = nc.declare_dram_parameter("bq", [D], F32, isOutput=False)
    wk = nc.declare_dram_parameter("Wk", [D, D], F32, isOutput=False)
    bk = nc.declare_dram_parameter("bk", [D], F32, isOutput=False)
    wv = nc.declare_dram_parameter("Wv", [D, D], F32, isOutput=False)
    bv = nc.declare_dram_parameter("bv", [D], F32, isOutput=False)
    out = nc.declare_dram_parameter("out", [BPC, S, D], F32, isOutput=True)

    with tile.TileContext(nc) as tc:
        with (
            tc.tile_pool(name="singles", bufs=1) as singles,
            tc.tile_pool(name="wst", bufs=2) as wst_pool,
            tc.tile_pool(name="xnat", bufs=2) as xnat_pool,
            tc.tile_pool(name="xc", bufs=2) as xc_pool,
            tc.tile_pool(name="xt", bufs=2) as xt_pool,
            tc.tile_pool(name="qt", bufs=2) as qt_pool,
            tc.tile_pool(name="kt", bufs=2) as kt_pool,
            tc.tile_pool(name="v", bufs=2) as v_pool,
            tc.tile_pool(name="pT", bufs=16) as pT_pool,
            tc.tile_pool(name="ostage", bufs=2) as o_pool,
            tc.tile_pool(name="rc", bufs=8) as rc_pool,
            tc.tile_pool(name="pssc", bufs=2, space="PSUM") as ps_sc,
            tc.tile_pool(name="pspr", bufs=1, space="PSUM") as ps_pr,
            tc.tile_pool(name="psctx", bufs=2, space="PSUM") as ps_ctx,
        ):
            # --- constants ---
            identity = singles.tile([128, 128], BF16)
            make_identity(nc, identity)
            # per-dout-tile bias columns: bqt[:, m] = bq[128m : 128(m+1)]
            bqt = singles.tile([128, ND], F32)
            bkt = singles.tile([128, ND], F32)
            nc.gpsimd.dma_start(out=bqt, in_=bq[:].rearrange("(m p) -> p m", p=128))
            nc.gpsimd.dma_start(out=bkt, in_=bk[:].rearrange("(m p) -> p m", p=128))
            # bv broadcast to all 128 partitions
            bvb = singles.tile([128, D], F32)
            bv_ap = bv[:]
            nc.gpsimd.dma_start(
                out=bvb,
                in_=bass.AP(tensor=bv_ap.tensor, offset=bv_ap.offset, ap=[[0, 128]] + bv_ap.ap),
            )

            # --- weights: loaded f32 once (gpsimd SWDGE queue — keeps them off
            # the sync queue that feeds X tiles and off the ACT queue that
            # runs the exps), cast to bf16, kept resident ---
            wres = {}

            def emit_wload(wname, wdram):
                tiles = []
                for k in range(ND):
                    wfull = wst_pool.tile([128, D], F32, tag="wst")
                    nc.gpsimd.dma_start(out=wfull, in_=wdram[ts(k, 128), :])
                    wb = singles.tile([128, D], BF16, tag=f"{wname}{k}", name=f"{wname}{k}")
                    nc.vector.tensor_copy(out=wb, in_=wfull)
                    tiles.append(wb)
                wres[wname] = tiles

            # ------------------------------------------------------------------
            # Software-pipelined emission. Per-engine queues are strict FIFO,
            # so program order IS the PE instruction order: interleave batch
            # b's QKV-projection matmuls (dense PE work) between batch b-1's
            # scores (which pace on ACT exp) and ctx matmuls. This keeps the
            # PE MM duty cycle high so the HAM clock gate stays at K=8/8.
            # ------------------------------------------------------------------
            state = {}  # per-batch tiles: xt3, qt3, kt3, vt, ost, ptiles

            def emit_ph0(b):
                st_ = {}
                xt_all = xt_pool.tile([128, ND * S], BF16, tag="xt", name="xt")
                st_["xt3"] = xt3 = xt_all.rearrange("p (k c) -> p k c", c=S)
                for t, (t0, stt) in enumerate(S_TILES):
                    xn = xnat_pool.tile([128, D], F32, tag="xn")
                    # batch 0 is latency-critical at startup: split X loads
                    # across the sync and scalar HWDGE queues
                    eng = nc.scalar if (b == 0 and t % 2) else nc.sync
                    eng.dma_start(out=xn[:stt], in_=hidden[b, t0 : t0 + stt, :])
                    xc = xc_pool.tile([128, D], BF16, tag="xc")
                    nc.vector.tensor_copy(out=xc[:stt], in_=xn[:stt])
                    # 8 bf16 transposes share one PSUM tile, single evac
                    pst = ps_pr.tile([128, 1024], BF16, tag="pr", name="pstr")
                    for j in range(ND):
                        nc.tensor.transpose(
                            pst[:, 128 * j : 128 * j + stt],
                            xc[:stt, ts(j, 128)],
                            identity[:stt, :stt],
                        )
                    src = pst.rearrange("p (j c) -> p j c", c=128)[:, :, 0:stt]
                    nc.vector.tensor_copy(out=xt3[:, :, t0 : t0 + stt], in_=src)
                qt_all = qt_pool.tile([128, ND * S], BF16, tag="qt", name="qt")
                kt_all = kt_pool.tile([128, ND * S], BF16, tag="kt", name="kt")
                st_["qt3"] = qt_all.rearrange("p (m c) -> p m c", c=S)
                st_["kt3"] = kt_all.rearrange("p (m c) -> p m c", c=S)
                st_["vt"] = [
                    v_pool.tile([128, H * 65], BF16, tag=f"v{t}", name=f"vtile{t}")
                    for t in range(len(S_TILES))
                ]
                st_["ost"] = [
                    o_pool.tile([128, D], F32, tag=f"o{j}", name=f"otile{j}")
                    for j in range(len(S_TILES))
                ]
                st_["ptiles"] = {}
                state[b] = st_

            def emit_proj_unit(b, u):
                st_ = state[b]
                xt3 = st_["xt3"]
                if u < 2 * ND:  # Q (u<8) or K (u<16) dout-tile m
                    wtiles, dst3, bias_t = (
                        (wres["wq"], st_["qt3"], bqt)
                        if u < ND
                        else (wres["wk"], st_["kt3"], bkt)
                    )
                    m = u % ND
                    ps = ps_pr.tile([128, 1024], F32, tag="pr", name="pspr")
                    for k in range(ND):
                        nc.tensor.matmul(
                            ps[:, 0:512], wtiles[k][:, ts(m, 128)], xt3[:, k, 0:512],
                            start=(k == 0), stop=(k == ND - 1),
                        )
                        nc.tensor.matmul(
                            ps[:, 512:S], wtiles[k][:, ts(m, 128)], xt3[:, k, 512:S],
                            start=(k == 0), stop=(k == ND - 1),
                        )
                    nc.vector.tensor_scalar_add(
                        dst3[:, m, :], ps[:, 0:S], bias_t[:, m : m + 1]
                    )
                else:  # V token-tile t
                    t = u - 2 * ND
                    t0, stt = S_TILES[t]
                    ps = ps_pr.tile([128, 1024], F32, tag="pr", name="pspr")
                    for k in range(ND):
                        nc.tensor.matmul(
                            ps[:stt, 0:512], xt3[:, k, t0 : t0 + stt], wres["wv"][k][:, 0:512],
                            start=(k == 0), stop=(k == ND - 1),
                        )
                        nc.tensor.matmul(
                            ps[:stt, 512:1024], xt3[:, k, t0 : t0 + stt], wres["wv"][k][:, 512:1024],
                            start=(k == 0), stop=(k == ND - 1),
                        )
                    v3 = st_["vt"][t].rearrange("p (h c) -> p h c", c=65)
                    nc.vector.tensor_copy(
                        out=v3[:stt, :, 0:64],
                        in_=ps[:stt].rearrange("p (h c) -> p h c", c=64),
                    )
                    nc.vector.memset(v3[:, :, 64:65], 1.0)

            def emit_scores(b, p):
                st_ = state[b]
                qt3, kt3 = st_["qt3"], st_["kt3"]
                ptiles = ([], [])
                for t, (t0, stt) in enumerate(S_TILES):
                    for half in range(2):
                        h0 = half * 64
                        psS = ps_sc.tile([128, 1024], F32, tag="sc", name="pssc")
                        nc.tensor.matmul(
                            psS[:stt, 0:512],
                            kt3[h0 : h0 + 64, p, t0 : t0 + stt],
                            qt3[h0 : h0 + 64, p, 0:512],
                            start=True, stop=True, tile_position=(h0, 0),
                        )
                        nc.tensor.matmul(
                            psS[:stt, 512:S],
                            kt3[h0 : h0 + 64, p, t0 : t0 + stt],
                            qt3[h0 : h0 + 64, p, 512:S],
                            start=True, stop=True, tile_position=(h0, 0),
                        )
                        pT = pT_pool.tile([128, S], BF16, tag="pT", name="pTtile")
                        nc.scalar.activation(pT[:stt], psS[:stt, 0:S], AF.Exp, scale=0.125)
                        ptiles[half].append(pT)
                st_["ptiles"][p] = ptiles

            def emit_ctx(b, p):
                st_ = state[b]
                ptiles = st_["ptiles"].pop(p)
                vt, ost = st_["vt"], st_["ost"]
                for half in range(2):
                    h = 2 * p + half
                    psc = ps_ctx.tile([128, 512], F32, tag="ctx", name="psctx")
                    for j, (j0, sj) in enumerate(S_TILES):
                        for t, (t0, stt) in enumerate(S_TILES):
                            nc.tensor.matmul(
                                psc[:sj, ds(65 * j, 65)],
                                ptiles[half][t][:stt, j0 : j0 + sj],
                                vt[t][:stt, ds(65 * h, 65)],
                                start=(t == 0), stop=(t == len(S_TILES) - 1),
                            )
                    # batched reciprocal of the 5 denominator columns
                    rc = rc_pool.tile([128, 8], F32, tag="rc", name="rctile")
                    psc3 = psc[:, 0:325].rearrange("p (j c) -> p j c", c=65)
                    rc3 = rc.rearrange("p (j c) -> p j c", c=1)
                    nc.vector.reciprocal(rc3[:, 0:5, :], psc3[:, 0:5, 64:65])
                    for j, (j0, sj) in enumerate(S_TILES):
                        nc.vector.scalar_tensor_tensor(
                            out=ost[j][:sj, ds(64 * h, 64)],
                            in0=psc[:sj, ds(65 * j, 64)],
                            scalar=rc[:sj, j : j + 1],
                            in1=bvb[:sj, ds(64 * h, 64)],
                            op0=OP.mult,
                            op1=OP.add,
                        )

            def emit_store(b):
                # gpsimd SWDGE queue: keeps the sync queue free for the next
                # batch's X loads (stores ahead of X loads in one FIFO caused
                # ~7us PE stalls at every batch boundary)
                ost = state[b]["ost"]
                for j, (j0, sj) in enumerate(S_TILES):
                    nc.gpsimd.dma_start(out=out[b, j0 : j0 + sj, :], in_=ost[j][:sj])
                del state[b]

            # Unit schedule: batch b's projection units [Q0-4, K0-4, V0-4]
            # fill iteration b's pair slots; its [Q5-7, K5-7] are deferred to
            # iteration b+1 slots 2-4 (before their consumers scores(b, 5-7)
            # at slots 5-7) so the final iteration — which has no next batch —
            # still gets dense PE filler and the HAM clock gate stays warm.
            MAIN_U = [0, 1, 2, 3, 4, 8, 9, 10, 11, 12, 16, 17, 18, 19, 20]
            DEFER_U = {2: [5, 13], 3: [6, 14], 4: [7, 15]}  # slot -> units
            NM = len(MAIN_U)
            NSLOT = HPAIRS + 1  # pair slots incl. ctx flush
            emit_ph0(0)
            emit_wload("wq", wq)
            emit_wload("wk", wk)
            emit_wload("wv", wv)
            for b in range(BPC + 1):
                for p in range(NSLOT):
                    if b >= 1 and p < HPAIRS:
                        emit_scores(b - 1, p)
                    if b >= 1 and p in DEFER_U:
                        for u in DEFER_U[p]:
                            emit_proj_unit(b - 1, u)
                    if b < BPC:
                        for ui in range(NM * p // NSLOT, NM * (p + 1) // NSLOT):
                            emit_proj_unit(b, MAIN_U[ui])
                    if p == 5 and b + 1 < BPC:
                        # hoist next batch's load/cast/transpose: its DVE work
                        # queues ahead of this iteration's tail ctx evacs, so
                        # the PE transposes aren't left waiting at the boundary
                        emit_ph0(b + 1)
                    if b >= 1 and p >= 1:
                        emit_ctx(b - 1, p - 1)
                if b >= 1:
                    emit_store(b - 1)

    return nc


_NC = None


def kernel(hidden_states, Wq, bq, Wk, bk, Wv, bv):
    global _NC
    if _NC is None:
        _NC = build_nc()
    hs = np.ascontiguousarray(np.asarray(hidden_states, dtype=np.float32))
    args = {
        "Wq": np.ascontiguousarray(np.asarray(Wq, np.float32)),
        "bq": np.ascontiguousarray(np.asarray(bq, np.float32)),
        "Wk": np.ascontiguousarray(np.asarray(Wk, np.float32)),
        "bk": np.ascontiguousarray(np.asarray(bk, np.float32)),
        "Wv": np.ascontiguousarray(np.asarray(Wv, np.float32)),
        "bv": np.ascontiguousarray(np.asarray(bv, np.float32)),
    }
    in_maps = [
        {"hidden": hs[i * BPC : (i + 1) * BPC], **args} for i in range(N_CORES)
    ]
    res = run_bass_kernel_spmd(_NC, in_maps, list(range(N_CORES)))
    return np.concatenate([res.results[i]["out"] for i in range(N_CORES)], axis=0)
